# revision 23
# baseline (speedup 1.0000x reference)
"""GQA attention prefill kernel for 8 Trainium2 NeuronCores.

Sharding: data-parallel over batch (2) x tensor-parallel over kv-heads
(4 groups of 2 kv-heads + their 8 q-heads). Each core computes its
partial out = attn_shard @ wo_shard; host sums the 4 row-parallel
partials per batch.

v2 layout strategy vs the fp32r baseline:
- x is transposed AND cast to bf16 on the host; the device loads x^T
  directly (no on-chip fp32 PE transposes, half the DMA bytes).
- All matmuls run bf16 (full PE rate, ~1e-2 headroom under the 2e-2
  error gate). Weights are host-packed per-core into the exact
  [128, k, m] SBUF layouts so every DMA is wide and contiguous.
- RoPE via host-permuted wq/wk columns (even dims -> partitions 0..63,
  odd -> 64..127), applied on DVE straight out of PSUM.
- Scores are computed transposed ([t, s]) into 2-bank-wide PSUM tiles;
  exp runs on ACT in [128, 1024] swaths; softmax denominator from
  bf16 DVE partial sums + a 1-row ones matmul; P@V needs no transposes.
- Software-pipelined emission: attention units (scores+exp+PV) for
  head h are interleaved between the projection matmuls of head h+2,
  so ACT exp work hides completely under PE projection work.
"""
import numpy as np
import ml_dtypes
from contextlib import ExitStack

import concourse.bass as bass
import concourse.tile as tile
from concourse import bacc, mybir
from concourse.bass_utils import run_bass_kernel_spmd
from concourse.masks import make_identity

dt = mybir.dt

DIM = 4096
N_HEADS = 32
N_KV = 8
HD = 128
B = 2
S = 1024
NCORES = 8
HPC = 8    # q-heads per core
KVPC = 2   # kv-heads per core
P = 128
SC = 512   # token chunk size
NKT = DIM // P      # 32 k-tiles over DIM
NTT = S // P        # 8 token tiles
NCH = S // SC       # 2 chunks
NOT = KVPC * 2 + HPC  # 12 projection out-tiles: K0 K1 V0 V1 Q0..Q7
SCALE = 1.0 / np.sqrt(HD)

_CACHE = {}


def _build():
    nc = bacc.Bacc("TRN2", target_bir_lowering=False, debug=False,
                   num_devices=NCORES)
    # host-packed inputs (see _host_prep for layouts)
    xt_d = nc.dram_tensor("xt", [P, NKT, S], dt.bfloat16, kind="ExternalInput").ap()
    w_d = nc.dram_tensor("wqk", [(KVPC + HPC) * P, NKT, P], dt.bfloat16,
                         kind="ExternalInput").ap()
    wv_d = nc.dram_tensor("wv", [P, NKT, KVPC * HD], dt.bfloat16,
                          kind="ExternalInput").ap()
    wo_d = nc.dram_tensor("wo", [P, HPC, DIM], dt.bfloat16, kind="ExternalInput").ap()
    cos_d = nc.dram_tensor("cos2", [P, S], dt.bfloat16, kind="ExternalInput").ap()
    sin_d = nc.dram_tensor("sinpm", [P, S], dt.bfloat16, kind="ExternalInput").ap()
    out_d = nc.dram_tensor("out", [DIM, S], dt.float32, kind="ExternalOutput").ap()

    with tile.TileContext(nc) as tc:
        with ExitStack() as ctx:
            persist = ctx.enter_context(tc.tile_pool(name="persist", bufs=1))
            ps_mm = ctx.enter_context(tc.tile_pool(name="ps_mm", bufs=2, space="PSUM"))
            ps_sw = ctx.enter_context(tc.tile_pool(name="ps_sw", bufs=2, space="PSUM"))
            ps_pv = ctx.enter_context(tc.tile_pool(name="ps_pv", bufs=2, space="PSUM"))
            wpool = ctx.enter_context(tc.tile_pool(name="wpool", bufs=3))
            wopool = ctx.enter_context(tc.tile_pool(name="wopool", bufs=2))
            rtmp = ctx.enter_context(tc.tile_pool(name="rtmp", bufs=2))
            vtp = ctx.enter_context(tc.tile_pool(name="vtp", bufs=2))
            epool = ctx.enter_context(tc.tile_pool(name="epool", bufs=3))
            spool = ctx.enter_context(tc.tile_pool(name="spool", bufs=2))
            opool = ctx.enter_context(tc.tile_pool(name="opool", bufs=4))

            # dependency-light PE warm-up: matmuls on a memset tile start
            # ~1us in and keep PE busy until the first weight/x DMAs land
            warm_src = persist.tile([P, SC], dt.bfloat16, tag="warm_src")
            nc.gpsimd.memset(warm_src[:], 1.0)
            warm_a = ps_mm.tile([P, SC], dt.float32, tag="mm", name="warm_a")
            warm_b = ps_mm.tile([P, SC], dt.float32, tag="mm", name="warm_b")
            for i in range(12):
                w_t = warm_a if i % 2 == 0 else warm_b
                nc.tensor.matmul(w_t[:], warm_src[:, 0:P], warm_src[:],
                                 start=True, stop=True)

            ones_f = persist.tile([P, 1], dt.float32, tag="ones_f")
            nc.gpsimd.memset(ones_f[:], 1.0)
            ones_b = persist.tile([P, 1], dt.bfloat16, tag="ones_b")
            nc.scalar.copy(ones_b[:], ones_f[:])

            cos2 = persist.tile([P, S], dt.bfloat16, tag="cos2")
            sinpm = persist.tile([P, S], dt.bfloat16, tag="sinpm")

            # Persistent activations
            xT = persist.tile([P, NKT, S], dt.bfloat16, tag="xT")
            qt = [persist.tile([P, S], dt.bfloat16, tag=f"qa{h}", name=f"qt{h}")
                  for h in range(HPC)]
            kt = persist.tile([P, KVPC, S], dt.bfloat16, tag="kt")
            vnat = persist.tile([P, NTT, KVPC * HD], dt.bfloat16, tag="v")

            # x^T arrives as 8 x 1MB transfers: chunk-major, 8 k-tiles each
            def dma_x(c, g):
                nc.sync.dma_start(
                    xT[:, g * 8:(g + 1) * 8, c * SC:(c + 1) * SC],
                    xt_d[:, g * 8:(g + 1) * 8, c * SC:(c + 1) * SC])

            w_tiles = {}

            def dma_w(ot):
                wsb = wpool.tile([P, NKT, P], dt.bfloat16, tag="w", name=f"w{ot}")
                nc.sync.dma_start(wsb[:], w_d[ot * P:(ot + 1) * P])
                w_tiles[ot] = wsb

            # startup order: w0 + the first half-chunk of x arrive first so
            # the opening K0 projection can start ~8us in; the K0.c0 chunk
            # is split into two N=256 passes to track DMA arrival.
            dma_w(0)
            nc.sync.dma_start(xT[:, 0:16, 0:256], xt_d[:, 0:16, 0:256])
            nc.sync.dma_start(xT[:, 16:NKT, 0:256], xt_d[:, 16:NKT, 0:256])
            nc.sync.dma_start(xT[:, 0:16, 256:SC], xt_d[:, 0:16, 256:SC])
            nc.sync.dma_start(xT[:, 16:NKT, 256:SC], xt_d[:, 16:NKT, 256:SC])
            dma_w(1)
            nc.sync.dma_start(cos2[:], cos_d[:])
            nc.sync.dma_start(sinpm[:], sin_d[:])
            wv_sb = persist.tile([P, NKT, KVPC * HD], dt.bfloat16, tag="wv")
            nc.sync.dma_start(wv_sb[:], wv_d[:])

            wo_tiles = {}

            def dma_wo(cc):
                wosb = wopool.tile([P, HPC, SC], dt.bfloat16, tag="wo",
                                   name=f"wo{cc}")
                nc.sync.dma_start(wosb[:], wo_d[:, :, cc * SC:(cc + 1) * SC])
                wo_tiles[cc] = wosb

            def rope_evict(psum, dest_ap, c, nm):
                # NB: the half-swapped muls must keep their misaligned
                # operand in PSUM (SBUF-SBUF partition-start mismatch is
                # rejected by the bir verifier)
                t0 = c * SC
                t1 = rtmp.tile([P, SC], dt.bfloat16, tag="t1")
                t2 = rtmp.tile([P, SC], dt.bfloat16, tag="t2")
                nc.vector.tensor_mul(out=t1[:], in0=psum[:], in1=cos2[:, t0:t0 + SC])
                nc.vector.tensor_mul(out=t2[0:64, :], in0=psum[64:P, :],
                                     in1=sinpm[0:64, t0:t0 + SC])
                nc.vector.tensor_mul(out=t2[64:P, :], in0=psum[0:64, :],
                                     in1=sinpm[64:P, t0:t0 + SC])
                nc.vector.tensor_add(out=dest_ap, in0=t1[:], in1=t2[:])

            # ---- projection chunk: psum += w[ot]^T @ xT[:, :, chunk] ----
            # emitted in two 16-matmul segments so attention-unit work can
            # slot in between without stalling PE on psum slots.
            def proj_seg(pq, ot, c, k0, k1):
                wsb = w_tiles[ot]
                for k in range(k0, k1):
                    nc.tensor.matmul(pq[:], wsb[:, k], xT[:, k, c * SC:(c + 1) * SC],
                                     start=(k == 0), stop=(k == NKT - 1))

            def proj_evict(pq, ot, c):
                if ot < KVPC:            # K head
                    rope_evict(pq, kt[:, ot, c * SC:(c + 1) * SC], c, f"k{ot}_{c}")
                else:                    # Q head
                    h = ot - KVPC
                    rope_evict(pq, qt[h][:, c * SC:(c + 1) * SC], c, f"q{h}_{c}")

            def proj_chunk(ot, c):
                pq = ps_mm.tile([P, SC], dt.float32, tag="mm", name=f"p{ot}_{c}")
                proj_seg(pq, ot, c, 0, NKT // 2)
                proj_seg(pq, ot, c, NKT // 2, NKT)
                proj_evict(pq, ot, c)

            # ---- V computed directly in [token, dim] layout: x^T tiles
            # stationary, wv moving -> no transposes, straight ACT evict.
            def v_chunk(c):
                for tj in range(SC // P):
                    tt = c * (SC // P) + tj
                    pv = ps_mm.tile([P, KVPC * HD], dt.float32, tag="mm",
                                    name=f"v{tt}")
                    for k in range(NKT):
                        nc.tensor.matmul(
                            pv[:], xT[:, k, tt * P:(tt + 1) * P], wv_sb[:, k],
                            start=(k == 0), stop=(k == NKT - 1))
                    nc.scalar.copy(vnat[:, tt, :], pv[:])

            # ---- attention unit (h, c): scores -> exp -> denom -> PV ----
            # returns emission callbacks so projection segments interleave.
            attn_tiles = {}

            def attn_unit(h, c):
                kv = h // 4
                e = epool.tile([P, NTT, SC], dt.bfloat16, tag="e", name=f"e{h}_{c}")
                part = spool.tile([P, SC], dt.bfloat16, tag="part",
                                  name=f"part{h}_{c}")
                sw_tiles = []

                def fill_wide(wi):
                    sw = ps_sw.tile([P, 2, SC], dt.float32, tag="sw",
                                    name=f"sw{h}_{c}_{wi}")
                    sw_tiles.append(sw)
                    for j in range(2):
                        tt = wi * 2 + j
                        nc.tensor.matmul(sw[:, j, :],
                                         kt[:, kv, tt * P:(tt + 1) * P],
                                         qt[h][:, c * SC:(c + 1) * SC],
                                         start=True, stop=True)
                    nc.scalar.activation(e[:, wi * 2:(wi + 1) * 2, :], sw[:],
                                         mybir.ActivationFunctionType.Exp,
                                         scale=float(SCALE))
                    # running bf16 denominator partials on DVE (4x mode)
                    if wi == 0:
                        nc.vector.tensor_add(out=part[:], in0=e[:, 0, :],
                                             in1=e[:, 1, :])
                    else:
                        for j in range(2):
                            nc.vector.tensor_add(out=part[:], in0=part[:],
                                                 in1=e[:, wi * 2 + j, :])

                def pv_and_norm():
                    po = ps_pv.tile([P, SC], dt.float32, tag="pv",
                                    name=f"pv{h}_{c}")
                    for tt in range(NTT):
                        nc.tensor.matmul(po[:], vnat[:, tt, kv * HD:(kv + 1) * HD],
                                         e[:, tt, :],
                                         start=(tt == 0), stop=(tt == NTT - 1))
                    pss = ps_mm.tile([1, SC], dt.float32, tag="mm",
                                     name=f"pss{h}_{c}")
                    nc.tensor.matmul(pss[:], ones_b[:], part[:],
                                     start=True, stop=True)
                    rrow = spool.tile([1, SC], dt.float32, tag="rrow")
                    nc.vector.reciprocal_approx_fast(rrow[:], pss[:])
                    rcb = spool.tile([P, SC], dt.float32, tag="rcb")
                    nc.gpsimd.partition_broadcast(rcb[:], rrow[:])
                    if h not in attn_tiles:
                        attn_tiles[h] = persist.tile([P, S], dt.bfloat16,
                                                     tag=f"qa{h}", name=f"attn{h}")
                    nc.vector.tensor_mul(out=attn_tiles[h][:, c * SC:(c + 1) * SC],
                                         in0=po[:], in1=rcb[:])

                return fill_wide, pv_and_norm

            # =========== emission schedule ===========
            # B-only prefix, chunk-0 work first (chunk-1 x is still landing):
            # K0.c0 K1.c0 V.c0 Q0.c0 then the same for chunk 1
            for g in range(4):
                dma_x(1, g)
            dma_w(2)
            for c in range(NCH):
                if c == 0:
                    # K0.c0 in two half-token passes tracking DMA arrival
                    pq0 = ps_mm.tile([P, SC], dt.float32, tag="mm", name="p0_0")
                    wsb0 = w_tiles[0]
                    for half in range(2):
                        t0 = half * 256
                        for k in range(NKT):
                            nc.tensor.matmul(
                                pq0[:, t0:t0 + 256], wsb0[:, k],
                                xT[:, k, t0:t0 + 256],
                                start=(k == 0), stop=(k == NKT - 1))
                    proj_evict(pq0, 0, 0)
                else:
                    proj_chunk(0, c)
                proj_chunk(1, c)
                v_chunk(c)
                proj_chunk(2, c)        # Q0
                if c == 0:
                    dma_w(3)
                    dma_w(4)
            w_tiles.pop(0)
            w_tiles.pop(1)
            w_tiles.pop(2)

            # interleaved: unit (h, c) paired with spacer chunk Q_{h+1}.c
            units = [(h, c) for h in range(HPC) for c in range(NCH)]
            for u, (h, c) in enumerate(units):
                fill_wide, pv_and_norm = attn_unit(h, c)
                if u < 14:
                    ot = 3 + u // 2       # Q_{h+1} projection as spacer
                    sc_ = u % 2
                    if sc_ == 0 and ot + 2 < KVPC + HPC:
                        dma_w(ot + 2)
                    pq = ps_mm.tile([P, SC], dt.float32, tag="mm",
                                    name=f"p{ot}_{sc_}")
                    fill_wide(0)
                    fill_wide(1)
                    proj_seg(pq, ot, sc_, 0, NKT // 2)
                    fill_wide(2)
                    fill_wide(3)
                    proj_seg(pq, ot, sc_, NKT // 2, NKT)
                    proj_evict(pq, ot, sc_)
                    if sc_ == 1:
                        w_tiles.pop(ot)
                    pv_and_norm()
                elif u == 14:
                    # tail pair: S(7,0), S(7,1), P(7,0), P(7,1)
                    tail0 = (fill_wide, pv_and_norm)
                    fill_wide(0)
                    fill_wide(1)
                    fill_wide(2)
                    fill_wide(3)
                else:
                    for wi in range(4):
                        fill_wide(wi)
                    tail0[1]()
                    pv_and_norm()
                if u % 2 == 1:
                    dma_wo(u // 2)      # prefetch wo chunks through phase C

            # ---- Phase D: out projection, streaming results out ----
            # Each (cc, ct) fills one 2-bank-wide psum tile (both token
            # chunks) so evicts overlap the next fill with only 2 slots.
            for cc in range(DIM // SC):
                wosb = wo_tiles.pop(cc)
                for ct in range(SC // P):
                    pdw = ps_sw.tile([P, 2, SC], dt.float32, tag="sw",
                                     name=f"pd{cc}_{ct}")
                    for k in range(HPC):
                        for c2 in range(NCH):
                            nc.tensor.matmul(
                                pdw[:, c2, :],
                                wosb[:, k, ct * P:(ct + 1) * P],
                                attn_tiles[k][:, c2 * SC:(c2 + 1) * SC],
                                start=(k == 0), stop=(k == HPC - 1))
                    for c2 in range(NCH):
                        osb = opool.tile([P, SC], dt.float32, tag="o")
                        if c2 == 0:
                            nc.vector.tensor_copy(osb[:], pdw[:, c2, :])
                        else:
                            nc.scalar.copy(osb[:], pdw[:, c2, :])
                        nc.sync.dma_start(
                            out_d[cc * SC + ct * P: cc * SC + (ct + 1) * P,
                                  c2 * SC:(c2 + 1) * SC],
                            osb[:])

    nc.compile()
    return nc


def _get_nc():
    if "nc" not in _CACHE:
        _CACHE["nc"] = _build()
    return _CACHE["nc"]


def _host_prep(x, freqs_cos, freqs_sin, wq, wk, wv, wo):
    bf16 = ml_dtypes.bfloat16
    x = np.asarray(x, dtype=np.float32)
    wq = np.asarray(wq, dtype=np.float32)
    wk = np.asarray(wk, dtype=np.float32)
    wv = np.asarray(wv, dtype=np.float32)
    wo = np.asarray(wo, dtype=np.float32)
    perm = np.empty(HD, np.int64)
    perm[0:64] = 2 * np.arange(64)
    perm[64:HD] = 2 * np.arange(64) + 1
    wqp = wq.reshape(DIM, N_HEADS, HD)[:, :, perm]
    wkp = wk.reshape(DIM, N_KV, HD)[:, :, perm]
    cosT = np.asarray(freqs_cos, np.float32).T  # [64, S]
    sinT = np.asarray(freqs_sin, np.float32).T
    cos2 = np.ascontiguousarray(
        np.concatenate([cosT, cosT], axis=0)).astype(bf16)   # [128, S]
    sinpm = np.ascontiguousarray(
        np.concatenate([-sinT, sinT], axis=0)).astype(bf16)

    def pack_w(cols):
        # [4096, 128] -> [128, 32, 128]  (partition, k-tile, out-col)
        return cols.reshape(NKT, P, P).transpose(1, 0, 2)

    in_maps = []
    for core in range(NCORES):
        b, g = core // 4, core % 4
        # x^T packed [128, 32, 1024]: (p, k, t) = x[t, k*128+p]
        xt = np.ascontiguousarray(
            x[b].T.reshape(NKT, P, S).transpose(1, 0, 2)).astype(bf16)
        wlist = ([pack_w(wkp[:, KVPC * g + i, :]) for i in range(KVPC)] +
                 [pack_w(wqp[:, HPC * g + i, :]) for i in range(HPC)])
        wpack = np.ascontiguousarray(np.stack(wlist)).reshape(
            (KVPC + HPC) * P, NKT, P).astype(bf16)
        # wv for this group's 2 kv heads: [4096, 256] -> [128, 32, 256]
        wvg = wv[:, KVPC * HD * g: KVPC * HD * (g + 1)]
        wvp = np.ascontiguousarray(
            wvg.reshape(NKT, P, KVPC * HD).transpose(1, 0, 2)).astype(bf16)
        # wo rows for this group's 8 heads: [1024, 4096] -> [128, 8, 4096]
        wog = wo[HPC * HD * g: HPC * HD * (g + 1), :]
        wop = np.ascontiguousarray(
            wog.reshape(HPC, P, DIM).transpose(1, 0, 2)).astype(bf16)
        in_maps.append({
            "xt": np.ascontiguousarray(xt),
            "wqk": np.ascontiguousarray(wpack),
            "wv": wvp,
            "wo": np.ascontiguousarray(wop),
            "cos2": cos2,
            "sinpm": sinpm,
        })
    return in_maps


def kernel(x, freqs_cos, freqs_sin, mask, input_indexes, wq, wk, wv, wo,
           cache_k, cache_v, **_ignored):
    in_maps = _host_prep(x, freqs_cos, freqs_sin, wq, wk, wv, wo)
    nc = _get_nc()
    res = run_bass_kernel_spmd(nc, in_maps, core_ids=list(range(NCORES)))
    outs = [res.results[c]["out"] for c in range(NCORES)]
    out = np.empty((B, S, DIM), np.float32)
    for b in range(B):
        acc = outs[4 * b]
        for g in range(1, 4):
            acc = acc + outs[4 * b + g]
        out[b] = acc.T
    return out


# revision 29
# speedup vs baseline: 1.1492x; 1.1492x over previous
"""GQA attention prefill kernel for 8 Trainium2 NeuronCores.

Sharding: data-parallel over batch (2) x tensor-parallel over kv-heads
(4 groups of 2 kv-heads + their 8 q-heads). Each core computes its
partial out = attn_shard @ wo_shard; host sums the 4 row-parallel
partials per batch.

v2 layout strategy vs the fp32r baseline:
- x is transposed AND cast to bf16 on the host; the device loads x^T
  directly (no on-chip fp32 PE transposes, half the DMA bytes).
- All matmuls run bf16 (full PE rate, ~1e-2 headroom under the 2e-2
  error gate). Weights are host-packed per-core into the exact
  [128, k, m] SBUF layouts so every DMA is wide and contiguous.
- RoPE via host-permuted wq/wk columns (even dims -> partitions 0..63,
  odd -> 64..127), applied on DVE straight out of PSUM.
- Scores are computed transposed ([t, s]) into 2-bank-wide PSUM tiles;
  exp runs on ACT in [128, 1024] swaths; softmax denominator from
  bf16 DVE partial sums + a 1-row ones matmul; P@V needs no transposes.
- Software-pipelined emission: attention units (scores+exp+PV) for
  head h are interleaved between the projection matmuls of head h+2,
  so ACT exp work hides completely under PE projection work.
"""
import numpy as np
import ml_dtypes
from contextlib import ExitStack

import concourse.bass as bass
import concourse.tile as tile
from concourse import bacc, bass_isa, mybir
from concourse.bass_utils import run_bass_kernel_spmd
from concourse.masks import make_identity

dt = mybir.dt

DIM = 4096
N_HEADS = 32
N_KV = 8
HD = 128
B = 2
S = 1024
NCORES = 8
HPC = 8    # q-heads per core
KVPC = 2   # kv-heads per core
P = 128
SC = 512   # token chunk size
NKT = DIM // P      # 32 k-tiles over DIM
NTT = S // P        # 8 token tiles
NCH = S // SC       # 2 chunks
NOT = KVPC * 2 + HPC  # 12 projection out-tiles: K0 K1 V0 V1 Q0..Q7
SCALE = 1.0 / np.sqrt(HD)

_CACHE = {}


def _build():
    nc = bacc.Bacc("TRN2", target_bir_lowering=False, debug=False,
                   num_devices=NCORES)
    # host-packed inputs (see _host_prep for layouts)
    xt_d = nc.dram_tensor("xt", [P, NKT, S], dt.bfloat16, kind="ExternalInput").ap()
    w_d = nc.dram_tensor("wqk", [(KVPC + HPC) * P, NKT, P], dt.bfloat16,
                         kind="ExternalInput").ap()
    wv_d = nc.dram_tensor("wv", [P, NKT, KVPC * HD], dt.bfloat16,
                          kind="ExternalInput").ap()
    wo_d = nc.dram_tensor("wo", [P, HPC, DIM], dt.bfloat16, kind="ExternalInput").ap()
    cos_d = nc.dram_tensor("cos2", [P, S], dt.bfloat16, kind="ExternalInput").ap()
    sin_d = nc.dram_tensor("sinpm", [P, S], dt.bfloat16, kind="ExternalInput").ap()
    out_d = nc.dram_tensor("out", [DIM, S], dt.float32, kind="ExternalOutput").ap()

    with tile.TileContext(nc) as tc:
        with ExitStack() as ctx:
            persist = ctx.enter_context(tc.tile_pool(name="persist", bufs=1))
            ps_mm = ctx.enter_context(tc.tile_pool(name="ps_mm", bufs=2, space="PSUM"))
            ps_sw = ctx.enter_context(tc.tile_pool(name="ps_sw", bufs=2, space="PSUM"))
            ps_pv = ctx.enter_context(tc.tile_pool(name="ps_pv", bufs=2, space="PSUM"))
            wpool = ctx.enter_context(tc.tile_pool(name="wpool", bufs=3))
            wopool = ctx.enter_context(tc.tile_pool(name="wopool", bufs=2))
            rtmp = ctx.enter_context(tc.tile_pool(name="rtmp", bufs=2))
            vtp = ctx.enter_context(tc.tile_pool(name="vtp", bufs=2))
            epool = ctx.enter_context(tc.tile_pool(name="epool", bufs=3))
            spool = ctx.enter_context(tc.tile_pool(name="spool", bufs=2))
            opool = ctx.enter_context(tc.tile_pool(name="opool", bufs=4))

            # dependency-light PE warm-up: matmuls on a memset tile start
            # ~1us in and keep PE busy until the first weight/x DMAs land
            warm_src = persist.tile([P, SC], dt.bfloat16, tag="warm_src")
            nc.gpsimd.memset(warm_src[:], 1.0)
            warm_a = ps_mm.tile([P, SC], dt.float32, tag="mm", name="warm_a")
            warm_b = ps_mm.tile([P, SC], dt.float32, tag="mm", name="warm_b")
            for i in range(18):
                w_t = warm_a if i % 2 == 0 else warm_b
                nc.tensor.matmul(w_t[:], warm_src[:, 0:P], warm_src[:],
                                 start=True, stop=True)

            cos2 = persist.tile([P, S], dt.bfloat16, tag="cos2")
            sinpm = persist.tile([P, S], dt.bfloat16, tag="sinpm")

            # Persistent activations
            xT = persist.tile([P, NKT, S], dt.bfloat16, tag="xT")
            qt = [persist.tile([P, S], dt.bfloat16, tag=f"qa{h}", name=f"qt{h}")
                  for h in range(HPC)]
            kt = persist.tile([P, KVPC, S], dt.bfloat16, tag="kt")
            vnat = persist.tile([P, NTT, KVPC * HD], dt.bfloat16, tag="v")

            # x^T arrives as 8 x 1MB transfers: chunk-major, 8 k-tiles each
            def dma_x(c, g):
                nc.sync.dma_start(
                    xT[:, g * 8:(g + 1) * 8, c * SC:(c + 1) * SC],
                    xt_d[:, g * 8:(g + 1) * 8, c * SC:(c + 1) * SC])

            w_tiles = {}

            def dma_w(ot):
                wsb = wpool.tile([P, NKT, P], dt.bfloat16, tag="w", name=f"w{ot}")
                nc.sync.dma_start(wsb[:], w_d[ot * P:(ot + 1) * P])
                w_tiles[ot] = wsb

            # startup: the opening K0.c0 projection is gated on w0 + all of
            # chunk-0 x (~5MB, ~14us); warm-up matmuls cover the wait.
            dma_w(0)
            dma_x(0, 0)
            dma_x(0, 1)
            dma_w(1)
            dma_x(0, 2)
            dma_x(0, 3)
            nc.sync.dma_start(cos2[:], cos_d[:])
            nc.sync.dma_start(sinpm[:], sin_d[:])
            wv_sb = persist.tile([P, NKT, KVPC * HD], dt.bfloat16, tag="wv")
            nc.sync.dma_start(wv_sb[:], wv_d[:])

            wo_tiles = {}

            def dma_wo(cc):
                wosb = wopool.tile([P, HPC, SC], dt.bfloat16, tag="wo",
                                   name=f"wo{cc}")
                nc.sync.dma_start(wosb[:], wo_d[:, :, cc * SC:(cc + 1) * SC])
                wo_tiles[cc] = wosb

            def rope_evict(psum, dest_ap, c, nm):
                # NB: the half-swapped muls must keep their misaligned
                # operand in PSUM (SBUF-SBUF partition-start mismatch is
                # rejected by the bir verifier)
                t0 = c * SC
                t1 = rtmp.tile([P, SC], dt.bfloat16, tag="t1")
                t2 = rtmp.tile([P, SC], dt.bfloat16, tag="t2")
                nc.vector.tensor_mul(out=t1[:], in0=psum[:], in1=cos2[:, t0:t0 + SC])
                nc.vector.tensor_mul(out=t2[0:64, :], in0=psum[64:P, :],
                                     in1=sinpm[0:64, t0:t0 + SC])
                nc.vector.tensor_mul(out=t2[64:P, :], in0=psum[0:64, :],
                                     in1=sinpm[64:P, t0:t0 + SC])
                nc.vector.tensor_add(out=dest_ap, in0=t1[:], in1=t2[:])

            # ---- projection chunk: psum += w[ot]^T @ xT[:, :, chunk] ----
            # emitted in two 16-matmul segments so attention-unit work can
            # slot in between without stalling PE on psum slots.
            def proj_seg(pq, ot, c, k0, k1):
                wsb = w_tiles[ot]
                for k in range(k0, k1):
                    nc.tensor.matmul(pq[:], wsb[:, k], xT[:, k, c * SC:(c + 1) * SC],
                                     start=(k == 0), stop=(k == NKT - 1))

            def proj_evict(pq, ot, c):
                if ot < KVPC:            # K head
                    rope_evict(pq, kt[:, ot, c * SC:(c + 1) * SC], c, f"k{ot}_{c}")
                else:                    # Q head
                    h = ot - KVPC
                    rope_evict(pq, qt[h][:, c * SC:(c + 1) * SC], c, f"q{h}_{c}")

            def proj_chunk(ot, c):
                pq = ps_mm.tile([P, SC], dt.float32, tag="mm", name=f"p{ot}_{c}")
                proj_seg(pq, ot, c, 0, NKT // 2)
                proj_seg(pq, ot, c, NKT // 2, NKT)
                proj_evict(pq, ot, c)

            # ---- V computed directly in [token, dim] layout: x^T tiles
            # stationary, wv moving -> no transposes, straight ACT evict.
            def v_chunk(c):
                for tj in range(SC // P):
                    tt = c * (SC // P) + tj
                    pv = ps_mm.tile([P, KVPC * HD], dt.float32, tag="mm",
                                    name=f"v{tt}")
                    for k in range(NKT):
                        nc.tensor.matmul(
                            pv[:], xT[:, k, tt * P:(tt + 1) * P], wv_sb[:, k],
                            start=(k == 0), stop=(k == NKT - 1))
                    nc.scalar.copy(vnat[:, tt, :], pv[:])

            # ---- attention unit (h, c): scores -> exp -> denom -> PV ----
            # returns emission callbacks so projection segments interleave.
            attn_tiles = {}

            def attn_unit(h, c):
                kv = h // 4
                e = epool.tile([P, NTT, SC], dt.bfloat16, tag="e", name=f"e{h}_{c}")
                part = spool.tile([P, SC], dt.bfloat16, tag="part",
                                  name=f"part{h}_{c}")
                sw_tiles = []

                def fill_wide(wi):
                    sw = ps_sw.tile([P, 2, SC], dt.float32, tag="sw",
                                    name=f"sw{h}_{c}_{wi}")
                    sw_tiles.append(sw)
                    for j in range(2):
                        tt = wi * 2 + j
                        nc.tensor.matmul(sw[:, j, :],
                                         kt[:, kv, tt * P:(tt + 1) * P],
                                         qt[h][:, c * SC:(c + 1) * SC],
                                         start=True, stop=True)
                    nc.scalar.activation(e[:, wi * 2:(wi + 1) * 2, :], sw[:],
                                         mybir.ActivationFunctionType.Exp,
                                         scale=float(SCALE))
                    # running bf16 denominator partials on DVE (4x mode)
                    if wi == 0:
                        nc.vector.tensor_add(out=part[:], in0=e[:, 0, :],
                                             in1=e[:, 1, :])
                    else:
                        for j in range(2):
                            nc.vector.tensor_add(out=part[:], in0=part[:],
                                                 in1=e[:, wi * 2 + j, :])

                def pv_and_norm():
                    po = ps_pv.tile([P, SC], dt.float32, tag="pv",
                                    name=f"pv{h}_{c}")
                    for tt in range(NTT):
                        nc.tensor.matmul(po[:], vnat[:, tt, kv * HD:(kv + 1) * HD],
                                         e[:, tt, :],
                                         start=(tt == 0), stop=(tt == NTT - 1))
                    # softmax denominator entirely off the PE: gpsimd
                    # partition-reduce of the bf16 partials, then reciprocal
                    rs = spool.tile([P, SC], dt.float32, tag="rs")
                    nc.gpsimd.partition_all_reduce(
                        rs[:], part[:], channels=P,
                        reduce_op=bass_isa.ReduceOp.add)
                    rcb = spool.tile([P, SC], dt.float32, tag="rcb")
                    nc.vector.reciprocal_approx_fast(rcb[:], rs[:])
                    if h not in attn_tiles:
                        attn_tiles[h] = persist.tile([P, S], dt.bfloat16,
                                                     tag=f"qa{h}", name=f"attn{h}")
                    nc.vector.tensor_mul(out=attn_tiles[h][:, c * SC:(c + 1) * SC],
                                         in0=po[:], in1=rcb[:])

                return fill_wide, pv_and_norm

            # =========== emission schedule ===========
            # B-only prefix, chunk-0 work first (chunk-1 x is still landing):
            # K0.c0 K1.c0 V.c0 Q0.c0 then the same for chunk 1
            for g in range(4):
                dma_x(1, g)
            dma_w(2)
            for c in range(NCH):
                proj_chunk(0, c)
                proj_chunk(1, c)
                v_chunk(c)
                proj_chunk(2, c)        # Q0
                if c == 0:
                    dma_w(3)
                    dma_w(4)
            w_tiles.pop(0)
            w_tiles.pop(1)
            w_tiles.pop(2)

            # interleaved: unit (h, c) paired with spacer chunk Q_{h+1}.c
            units = [(h, c) for h in range(HPC) for c in range(NCH)]
            for u, (h, c) in enumerate(units):
                fill_wide, pv_and_norm = attn_unit(h, c)
                if u < 14:
                    ot = 3 + u // 2       # Q_{h+1} projection as spacer
                    sc_ = u % 2
                    if sc_ == 0 and ot + 2 < KVPC + HPC:
                        dma_w(ot + 2)
                    pq = ps_mm.tile([P, SC], dt.float32, tag="mm",
                                    name=f"p{ot}_{sc_}")
                    fill_wide(0)
                    fill_wide(1)
                    proj_seg(pq, ot, sc_, 0, NKT // 2)
                    fill_wide(2)
                    fill_wide(3)
                    proj_seg(pq, ot, sc_, NKT // 2, NKT)
                    proj_evict(pq, ot, sc_)
                    if sc_ == 1:
                        w_tiles.pop(ot)
                    pv_and_norm()
                elif u == 14:
                    # tail pair: S(7,0), S(7,1), P(7,0), P(7,1)
                    tail0 = (fill_wide, pv_and_norm)
                    fill_wide(0)
                    fill_wide(1)
                    fill_wide(2)
                    fill_wide(3)
                else:
                    for wi in range(4):
                        fill_wide(wi)
                    tail0[1]()
                    pv_and_norm()
                if u % 2 == 1:
                    dma_wo(u // 2)      # prefetch wo chunks through phase C

            # ---- Phase D: out projection, streaming results out ----
            # Each (cc, ct) fills one 2-bank-wide psum tile (both token
            # chunks) so evicts overlap the next fill with only 2 slots.
            for cc in range(DIM // SC):
                wosb = wo_tiles.pop(cc)
                for ct in range(SC // P):
                    pdw = ps_sw.tile([P, 2, SC], dt.float32, tag="sw",
                                     name=f"pd{cc}_{ct}")
                    for k in range(HPC):
                        for c2 in range(NCH):
                            nc.tensor.matmul(
                                pdw[:, c2, :],
                                wosb[:, k, ct * P:(ct + 1) * P],
                                attn_tiles[k][:, c2 * SC:(c2 + 1) * SC],
                                start=(k == 0), stop=(k == HPC - 1))
                    for c2 in range(NCH):
                        osb = opool.tile([P, SC], dt.float32, tag="o")
                        if c2 == 0:
                            nc.vector.tensor_copy(osb[:], pdw[:, c2, :])
                        else:
                            nc.scalar.copy(osb[:], pdw[:, c2, :])
                        nc.sync.dma_start(
                            out_d[cc * SC + ct * P: cc * SC + (ct + 1) * P,
                                  c2 * SC:(c2 + 1) * SC],
                            osb[:])

    nc.compile()
    return nc


def _get_nc():
    if "nc" not in _CACHE:
        _CACHE["nc"] = _build()
    return _CACHE["nc"]


def _host_prep(x, freqs_cos, freqs_sin, wq, wk, wv, wo):
    bf16 = ml_dtypes.bfloat16
    x = np.asarray(x, dtype=np.float32)
    wq = np.asarray(wq, dtype=np.float32)
    wk = np.asarray(wk, dtype=np.float32)
    wv = np.asarray(wv, dtype=np.float32)
    wo = np.asarray(wo, dtype=np.float32)
    perm = np.empty(HD, np.int64)
    perm[0:64] = 2 * np.arange(64)
    perm[64:HD] = 2 * np.arange(64) + 1
    wqp = wq.reshape(DIM, N_HEADS, HD)[:, :, perm]
    wkp = wk.reshape(DIM, N_KV, HD)[:, :, perm]
    cosT = np.asarray(freqs_cos, np.float32).T  # [64, S]
    sinT = np.asarray(freqs_sin, np.float32).T
    cos2 = np.ascontiguousarray(
        np.concatenate([cosT, cosT], axis=0)).astype(bf16)   # [128, S]
    sinpm = np.ascontiguousarray(
        np.concatenate([-sinT, sinT], axis=0)).astype(bf16)

    def pack_w(cols):
        # [4096, 128] -> [128, 32, 128]  (partition, k-tile, out-col)
        return cols.reshape(NKT, P, P).transpose(1, 0, 2)

    in_maps = []
    for core in range(NCORES):
        b, g = core // 4, core % 4
        # x^T packed [128, 32, 1024]: (p, k, t) = x[t, k*128+p]
        xt = np.ascontiguousarray(
            x[b].T.reshape(NKT, P, S).transpose(1, 0, 2)).astype(bf16)
        wlist = ([pack_w(wkp[:, KVPC * g + i, :]) for i in range(KVPC)] +
                 [pack_w(wqp[:, HPC * g + i, :]) for i in range(HPC)])
        wpack = np.ascontiguousarray(np.stack(wlist)).reshape(
            (KVPC + HPC) * P, NKT, P).astype(bf16)
        # wv for this group's 2 kv heads: [4096, 256] -> [128, 32, 256]
        wvg = wv[:, KVPC * HD * g: KVPC * HD * (g + 1)]
        wvp = np.ascontiguousarray(
            wvg.reshape(NKT, P, KVPC * HD).transpose(1, 0, 2)).astype(bf16)
        # wo rows for this group's 8 heads: [1024, 4096] -> [128, 8, 4096]
        wog = wo[HPC * HD * g: HPC * HD * (g + 1), :]
        wop = np.ascontiguousarray(
            wog.reshape(HPC, P, DIM).transpose(1, 0, 2)).astype(bf16)
        in_maps.append({
            "xt": np.ascontiguousarray(xt),
            "wqk": np.ascontiguousarray(wpack),
            "wv": wvp,
            "wo": np.ascontiguousarray(wop),
            "cos2": cos2,
            "sinpm": sinpm,
        })
    return in_maps


def kernel(x, freqs_cos, freqs_sin, mask, input_indexes, wq, wk, wv, wo,
           cache_k, cache_v, **_ignored):
    in_maps = _host_prep(x, freqs_cos, freqs_sin, wq, wk, wv, wo)
    nc = _get_nc()
    res = run_bass_kernel_spmd(nc, in_maps, core_ids=list(range(NCORES)))
    outs = [res.results[c]["out"] for c in range(NCORES)]
    out = np.empty((B, S, DIM), np.float32)
    for b in range(B):
        acc = outs[4 * b]
        for g in range(1, 4):
            acc = acc + outs[4 * b + g]
        out[b] = acc.T
    return out
